# revision 5
# baseline (speedup 1.0000x reference)
"""CondNet kernel for Trainium2 (8 NeuronCores, model-parallel mid layers).

Model (reference):
    h1 = relu(x @ W_in.T + b_in)                     # (512, 8192)
    h  = relu(condensed(h, W_mid[i], b_mid[i]))      # i in {0, 1}; gather + weighted sum
    out = h @ W_out.T + b_out                        # (512, 1000)

Strategy (v1, model-parallel):
  - The condensed (gather) layers are re-expressed as dense matmuls
    h @ S where S[k, o] = sum_f W_mid[o, f] * [indx_seqs[o, f] == k],
    built on the host (compile-time transform of the weights+indices).
  - Unlike the data-parallel v0 (every core streamed the FULL 134 MB
    S per layer -> DMA-bound), every core here keeps the FULL batch
    (512) and computes a 1/8 OUTPUT-slice of every layer, so it only
    reads its own 16.8 MB slice of each S.  Between layers an
    AllGather (1 MB/rank, ~14 us on-chip) reassembles the full
    activations.  This cuts per-core HBM traffic ~8x and the kernel
    becomes TensorEngine-bound (~110 us per condensed layer).
  - All activations flow k-major ([feature, batch]) so every layer's
    matmul is out[o_tile, b] = S_slab[k, o_tile].T @ h[k, b]: the
    S slab is the stationary operand, activations are moving, and no
    on-chip transposes are needed anywhere.
  - Weights bf16, accumulation in fp32 PSUM; bias+ReLU fused into the
    Scalar-engine PSUM->SBUF eviction (bias is per-partition).
  - Final b_out is added on the host (free; [1000] broadcast).
"""

import os
import numpy as np
import ml_dtypes

import concourse.bass as bass
import concourse.tile as tile
from concourse import bacc, mybir
from concourse import bass_utils

BF16 = ml_dtypes.bfloat16

B, NUM_IN, NUM_MID, NUM_OUT, FAN_IN, N_COND = 512, 1024, 8192, 1000, 64, 2
NCORES = 8
OSLAB = NUM_MID // NCORES      # 1024 outputs per core for mid layers
JSLAB = NUM_OUT // NCORES      # 125 final outputs per core (padded to 128)
RG = [list(range(NCORES))]     # one replica group: all 8 cores

_cache = {}
LAST_RESULT = None  # BassKernelResults of the most recent run (for test harness)


def _build_nc():
    """Build + compile the Bass program (same SPMD program for all 8 cores)."""
    nc = bacc.Bacc("TRN2", target_bir_lowering=False, debug=False, num_devices=NCORES)
    f32, bf16 = mybir.dt.float32, mybir.dt.bfloat16

    # ---- DRAM I/O (per-core slabs; k-major tiles everywhere) ----
    xT_d = nc.dram_tensor("xT", [8, 128, B], bf16, kind="ExternalInput").ap()
    w_in_d = nc.dram_tensor("w_in", [8, 128, OSLAB], bf16, kind="ExternalInput").ap()
    b_in_d = nc.dram_tensor("b_in", [128, 8], f32, kind="ExternalInput").ap()
    s1_d = nc.dram_tensor("s1", [64, 128, OSLAB], bf16, kind="ExternalInput").ap()
    b1_d = nc.dram_tensor("b1", [128, 8], f32, kind="ExternalInput").ap()
    s2_d = nc.dram_tensor("s2", [64, 128, OSLAB], bf16, kind="ExternalInput").ap()
    b2_d = nc.dram_tensor("b2", [128, 8], f32, kind="ExternalInput").ap()
    w_out_d = nc.dram_tensor("w_out", [64, 128, 128], bf16, kind="ExternalInput").ap()
    out_d = nc.dram_tensor("out", [128, B], f32, kind="ExternalOutput").ap()

    with tile.TileContext(nc) as tc:
        with (
            tc.tile_pool(name="const", bufs=1) as cpool,
            tc.tile_pool(name="sstream", bufs=12) as spool,
            tc.tile_pool(name="hstream", bufs=12) as hpool,
            tc.tile_pool(name="acts", bufs=4) as apool,
            tc.tile_pool(name="psmm", bufs=1, space="PSUM") as pmm,
            tc.tile_pool(name="dram", bufs=1, space="DRAM") as dpool,
        ):
            # persistent SBUF tensors
            xT = cpool.tile([128, 8, B], bf16)
            nc.sync.dma_start(out=xT[:], in_=xT_d.rearrange("u p b -> p u b"))
            w_in = cpool.tile([128, 8, OSLAB], bf16)
            nc.sync.dma_start(out=w_in[:], in_=w_in_d.rearrange("u p o -> p u o"))
            w_out = cpool.tile([128, 64, 128], bf16)
            nc.sync.dma_start(out=w_out[:], in_=w_out_d.rearrange("u p j -> p u j"))
            b_in = cpool.tile([128, 8], f32)
            b1 = cpool.tile([128, 8], f32)
            b2 = cpool.tile([128, 8], f32)
            nc.sync.dma_start(out=b_in[:], in_=b_in_d)
            nc.sync.dma_start(out=b1[:], in_=b1_d)
            nc.sync.dma_start(out=b2[:], in_=b2_d)

            # DRAM bounce buffers for the two AllGathers
            h1b = dpool.tile([OSLAB, B], bf16, tag="h1b")
            h1g = dpool.tile([NUM_MID, B], bf16, tag="h1g", addr_space="Shared")
            h2b = dpool.tile([OSLAB, B], bf16, tag="h2b")
            h2g = dpool.tile([NUM_MID, B], bf16, tag="h2g", addr_space="Shared")

            def mid_layer(h_gathered, s_dram, bias, out_bounce):
                """Full-batch dense layer: out slab (1024 cols) of h @ S.

                h_gathered: DRAM [8192, 512] bf16 (k-major rows)
                s_dram:     DRAM [64, 128, 1024] bf16 (stationary k-tiles)
                bias:       SBUF [128, 8] f32
                out_bounce: DRAM [1024, 512] bf16
                """
                psums = [pmm.tile([128, B], f32, tag=f"mm{ot}", name=f"ps{ot}") for ot in range(8)]
                for kt in range(64):
                    st = spool.tile([128, OSLAB], bf16, tag="s")
                    nc.sync.dma_start(out=st[:], in_=s_dram[kt])
                    ht = hpool.tile([128, B], bf16, tag="h")
                    nc.sync.dma_start(
                        out=ht[:], in_=h_gathered[kt * 128:(kt + 1) * 128, :])
                    for ot in range(8):
                        nc.tensor.matmul(
                            psums[ot][:],
                            st[:, ot * 128:(ot + 1) * 128],
                            ht[:],
                            start=(kt == 0),
                            stop=(kt == 63),
                        )
                for ot in range(8):
                    act = apool.tile([128, B], bf16, tag="act")
                    nc.scalar.activation(
                        act[:], psums[ot][:],
                        mybir.ActivationFunctionType.Relu,
                        bias=bias[:, ot:ot + 1],
                    )
                    nc.sync.dma_start(
                        out=out_bounce[ot * 128:(ot + 1) * 128, :], in_=act[:])

            # ---- L1: h1 slab = relu(W_in_slab.T @ xT + b_in) ----
            psums = [pmm.tile([128, B], f32, tag=f"mm{ot}", name=f"ps{ot}") for ot in range(8)]
            for kt in range(8):
                for ot in range(8):
                    nc.tensor.matmul(
                        psums[ot][:],
                        w_in[:, kt, ot * 128:(ot + 1) * 128],
                        xT[:, kt, :],
                        start=(kt == 0),
                        stop=(kt == 7),
                    )
            for ot in range(8):
                act = apool.tile([128, B], bf16, tag="act")
                nc.scalar.activation(
                    act[:], psums[ot][:],
                    mybir.ActivationFunctionType.Relu,
                    bias=b_in[:, ot:ot + 1],
                )
                nc.sync.dma_start(
                    out=h1b[ot * 128:(ot + 1) * 128, :], in_=act[:])

            nc.gpsimd.collective_compute(
                "AllGather", mybir.AluOpType.bypass, replica_groups=RG,
                ins=[h1b.opt()], outs=[h1g.opt()])

            mid_layer(h1g, s1_d, b1, h2b)

            nc.gpsimd.collective_compute(
                "AllGather", mybir.AluOpType.bypass, replica_groups=RG,
                ins=[h2b.opt()], outs=[h2g.opt()])

            h3b = dpool.tile([OSLAB, B], bf16, tag="h3b")
            h3g = dpool.tile([NUM_MID, B], bf16, tag="h3g", addr_space="Shared")
            mid_layer(h2g, s2_d, b2, h3b)

            nc.gpsimd.collective_compute(
                "AllGather", mybir.AluOpType.bypass, replica_groups=RG,
                ins=[h3b.opt()], outs=[h3g.opt()])

            # ---- L4: out slab (128 of 1024-padded) = W_out_slab.T @ h3 ----
            psum_o = pmm.tile([128, B], f32, tag="mm0")
            for kt in range(64):
                ht = hpool.tile([128, B], bf16, tag="h")
                nc.sync.dma_start(
                    out=ht[:], in_=h3g[kt * 128:(kt + 1) * 128, :])
                nc.tensor.matmul(
                    psum_o[:],
                    w_out[:, kt, :],
                    ht[:],
                    start=(kt == 0),
                    stop=(kt == 63),
                )
            osb = apool.tile([128, B], f32, tag="out")
            nc.vector.tensor_copy(osb[:], psum_o[:])
            nc.sync.dma_start(out=out_d, in_=osb[:])

    nc.compile()
    return nc


def _prep_inputs(x, W_in, b_in, W_mid, b_mid, W_out, b_out, indx_seqs):
    """Host-side compile-time transforms of inputs (per-core slabs)."""
    idx = np.asarray(indx_seqs).astype(np.int64)

    def build_S(Wm):
        # S[k, o] = sum_f Wm[o, f] * [idx[o, f] == k]
        S = np.zeros((NUM_MID, NUM_MID), np.float32)
        cols = np.repeat(np.arange(NUM_MID), FAN_IN)
        np.add.at(S, (idx.reshape(-1), cols), np.asarray(Wm, np.float32).reshape(-1))
        return S.reshape(64, 128, NUM_MID).astype(BF16)  # k-major tiles

    s1_t = build_S(W_mid[0])
    s2_t = build_S(W_mid[1])

    x = np.asarray(x, np.float32)
    xT = np.ascontiguousarray(x.T.reshape(8, 128, B).astype(BF16))
    w_in_t = np.asarray(W_in, np.float32).T.reshape(8, 128, NUM_MID).astype(BF16)

    woT = np.asarray(W_out, np.float32).T  # [8192, 1000]

    def bias_slab(b, c):
        return np.ascontiguousarray(
            np.asarray(b, np.float32)[c * OSLAB:(c + 1) * OSLAB].reshape(8, 128).T)

    in_maps = []
    for c in range(NCORES):
        sl = slice(c * OSLAB, (c + 1) * OSLAB)
        wo = np.zeros((NUM_MID, 128), np.float32)
        wo[:, :JSLAB] = woT[:, c * JSLAB:(c + 1) * JSLAB]
        in_maps.append({
            "xT": xT,
            "w_in": np.ascontiguousarray(w_in_t[:, :, sl]),
            "b_in": bias_slab(b_in, c),
            "s1": np.ascontiguousarray(s1_t[:, :, sl]),
            "b1": bias_slab(b_mid[0], c),
            "s2": np.ascontiguousarray(s2_t[:, :, sl]),
            "b2": bias_slab(b_mid[1], c),
            "w_out": np.ascontiguousarray(wo.reshape(64, 128, 128).astype(BF16)),
        })
    return in_maps, np.asarray(b_out, np.float32)


def kernel(x, W_in, b_in, W_mid, b_mid, W_out, b_out, indx_seqs):
    global LAST_RESULT
    if "nc" not in _cache:
        _cache["nc"] = _build_nc()
    nc = _cache["nc"]

    in_maps, b_out_f = _prep_inputs(x, W_in, b_in, W_mid, b_mid, W_out, b_out,
                                    indx_seqs)

    res = bass_utils.run_bass_kernel_spmd(
        nc, in_maps, core_ids=list(range(NCORES)),
        trace=bool(int(os.environ.get("KERNEL_TRACE", "0"))),
    )
    LAST_RESULT = res

    out = np.concatenate([r["out"][:JSLAB] for r in res.results], axis=0)  # [1000, B]
    return np.ascontiguousarray(out.T + b_out_f[None, :]).astype(np.float32)


# revision 6
# speedup vs baseline: 1.2534x; 1.2534x over previous
"""CondNet kernel for Trainium2 (8 NeuronCores, model-parallel mid layers).

v2 over v1 (v1 measured 491 us, of which 232 us HAM-throttled PE):
  - DMA issue split across both HWDGE rings: the 16.8 MB/layer S-slab
    stream goes on the SP (sync) ring, the 8.4 MB/layer activation
    stream on the Activation (scalar) ring, bounce-out copies and
    collectives on gpsimd (SWDGE).  v1 pushed everything through the
    sync ring, whose FIFO sequencing starved the PE every k-tile and
    kept re-triggering the HAM throttle (50% util cap).
  - Streams fetch 2 k-tiles per DMA (512 KB / 256 KB) to halve the
    per-DMA fixed costs.
  - Each AllGather is split into two 0.5 MB halves: the first half
    fires while the producing layer's second half is still computing
    (S rows are host-permuted to the half-gathered k-order).
  - No third AllGather: L4 (out = h3 @ W_out.T) is computed as
    per-core k-partials straight from L3's SBUF activation tiles and
    the 8 partials are summed on the host.
  - As v1: condensed layers as dense h @ S matmuls (S built on host
    from indx_seqs/W_mid), k-major activations, S slabs stationary /
    activations moving, bf16 data with fp32 PSUM accumulation,
    bias+ReLU fused into the Scalar-engine PSUM eviction.
"""

import os
import numpy as np
import ml_dtypes

import concourse.bass as bass
import concourse.tile as tile
from concourse import bacc, mybir
from concourse import bass_utils

BF16 = ml_dtypes.bfloat16

B, NUM_IN, NUM_MID, NUM_OUT, FAN_IN, N_COND = 512, 1024, 8192, 1000, 64, 2
NCORES = 8
OSLAB = NUM_MID // NCORES      # 1024 outputs per core for mid layers
HALF = OSLAB // 2              # 512-row AllGather halves
RG = [list(range(NCORES))]     # one replica group: all 8 cores

_cache = {}
LAST_RESULT = None  # BassKernelResults of the most recent run (for test harness)


def _build_nc():
    """Build + compile the Bass program (same SPMD program for all 8 cores)."""
    nc = bacc.Bacc("TRN2", target_bir_lowering=False, debug=False, num_devices=NCORES)
    f32, bf16 = mybir.dt.float32, mybir.dt.bfloat16

    # ---- DRAM I/O (per-core slabs; k-major tiles everywhere) ----
    xT_d = nc.dram_tensor("xT", [8, 128, B], bf16, kind="ExternalInput").ap()
    w_in_d = nc.dram_tensor("w_in", [8, 128, OSLAB], bf16, kind="ExternalInput").ap()
    b_in_d = nc.dram_tensor("b_in", [128, 8], f32, kind="ExternalInput").ap()
    s1_d = nc.dram_tensor("s1", [64, 128, OSLAB], bf16, kind="ExternalInput").ap()
    b1_d = nc.dram_tensor("b1", [128, 8], f32, kind="ExternalInput").ap()
    s2_d = nc.dram_tensor("s2", [64, 128, OSLAB], bf16, kind="ExternalInput").ap()
    b2_d = nc.dram_tensor("b2", [128, 8], f32, kind="ExternalInput").ap()
    w_out_d = nc.dram_tensor("w_out", [8, 128, 1024], bf16, kind="ExternalInput").ap()
    out_d = nc.dram_tensor("out", [8, 128, B], f32, kind="ExternalOutput").ap()

    with tile.TileContext(nc) as tc:
        with (
            tc.tile_pool(name="const", bufs=1) as cpool,
            tc.tile_pool(name="sstream", bufs=6) as spool,
            tc.tile_pool(name="hstream", bufs=6) as hpool,
            tc.tile_pool(name="acts", bufs=4) as apool,
            tc.tile_pool(name="psmm", bufs=1, space="PSUM") as pmm,
            tc.tile_pool(name="dram", bufs=1, space="DRAM") as dpool,
        ):
            # persistent SBUF tensors. xT/w_in on the sync ring (needed
            # first), w_out/biases on the scalar ring (idle at start).
            xT = cpool.tile([128, 8, B], bf16)
            nc.sync.dma_start(out=xT[:], in_=xT_d.rearrange("u p b -> p u b"))
            w_in = cpool.tile([128, 8, OSLAB], bf16)
            nc.sync.dma_start(out=w_in[:], in_=w_in_d.rearrange("u p o -> p u o"))
            w_out = cpool.tile([128, 8, 1024], bf16)
            nc.scalar.dma_start(out=w_out[:], in_=w_out_d.rearrange("u p j -> p u j"))
            b_in = cpool.tile([128, 8], f32)
            b1 = cpool.tile([128, 8], f32)
            b2 = cpool.tile([128, 8], f32)
            nc.scalar.dma_start(out=b_in[:], in_=b_in_d)
            nc.scalar.dma_start(out=b1[:], in_=b1_d)
            nc.scalar.dma_start(out=b2[:], in_=b2_d)

            # DRAM bounce buffers: per-half AllGathers (tile-major 3D so
            # sliced chunks can be partition-rearranged on the fly)
            h1b = [dpool.tile([4, 128, B], bf16, tag=f"h1b{i}", name=f"h1b{i}")
                   for i in range(2)]
            h1g = [dpool.tile([32, 128, B], bf16, tag=f"h1g{i}",
                              name=f"h1g{i}", addr_space="Shared")
                   for i in range(2)]
            h2b = [dpool.tile([4, 128, B], bf16, tag=f"h2b{i}", name=f"h2b{i}")
                   for i in range(2)]
            h2g = [dpool.tile([32, 128, B], bf16, tag=f"h2g{i}",
                              name=f"h2g{i}", addr_space="Shared")
                   for i in range(2)]

            def evict(psums, bias, bounce, keep_sbuf=False):
                """PSUM -> bias+ReLU -> SBUF bf16 -> (DRAM half-bounces + AG).

                Emits the half-AllGather right after each half's 4 tiles
                are out.  Returns the SBUF act tiles if keep_sbuf.
                """
                kept = []
                for half in range(2):
                    for i in range(4):
                        ot = half * 4 + i
                        if keep_sbuf:
                            act = cpool.tile([128, B], bf16, tag=f"h3k{ot}",
                                             name=f"h3k{ot}")
                        else:
                            act = apool.tile([128, B], bf16, tag="act",
                                             name="act")
                        nc.scalar.activation(
                            act[:], psums[ot][:],
                            mybir.ActivationFunctionType.Relu,
                            bias=bias[:, ot:ot + 1],
                        )
                        kept.append(act)
                        if bounce is not None:
                            nc.gpsimd.dma_start(
                                out=bounce[0][half][i], in_=act[:])
                    if bounce is not None:
                        nc.gpsimd.collective_compute(
                            "AllGather", mybir.AluOpType.bypass,
                            replica_groups=RG,
                            ins=[bounce[0][half].opt()],
                            outs=[bounce[1][half].opt()])
                return kept if keep_sbuf else None

            def mid_layer(h_halves, s_dram, bias, bounce, keep_sbuf=False):
                """Full-batch dense layer: out slab (1024 cols) of h @ S.

                h_halves: two DRAM [32, 128, 512] bf16 gathered halves
                          (k-order: S rows are host-permuted to match)
                """
                psums = [pmm.tile([128, B], f32, tag=f"mm{ot}", name=f"ps{ot}")
                         for ot in range(8)]
                for ck in range(32):           # 2 k-tiles per chunk
                    st = spool.tile([128, 2, OSLAB], bf16, tag="s", name="st")
                    nc.sync.dma_start(
                        out=st[:],
                        in_=s_dram[2 * ck:2 * ck + 2].rearrange("u p o -> p u o"))
                    src = h_halves[ck // 16]
                    lo = (ck % 16) * 2
                    ht = hpool.tile([128, 2, B], bf16, tag="h", name="ht")
                    nc.scalar.dma_start(
                        out=ht[:],
                        in_=src[lo:lo + 2].rearrange("u p b -> p u b"))
                    for u in range(2):
                        kt = 2 * ck + u
                        for ot in range(8):
                            nc.tensor.matmul(
                                psums[ot][:],
                                st[:, u, ot * 128:(ot + 1) * 128],
                                ht[:, u, :],
                                start=(kt == 0),
                                stop=(kt == 63),
                            )
                return evict(psums, bias, bounce, keep_sbuf)

            # ---- L1: h1 slab = relu(W_in_slab.T @ xT + b_in) ----
            psums = [pmm.tile([128, B], f32, tag=f"mm{ot}", name=f"ps{ot}")
                     for ot in range(8)]
            for kt in range(8):
                for ot in range(8):
                    nc.tensor.matmul(
                        psums[ot][:],
                        w_in[:, kt, ot * 128:(ot + 1) * 128],
                        xT[:, kt, :],
                        start=(kt == 0),
                        stop=(kt == 7),
                    )
            evict(psums, b_in, (h1b, h1g))

            mid_layer(h1g, s1_d, b1, (h2b, h2g))
            h3 = mid_layer(h2g, s2_d, b2, None, keep_sbuf=True)

            # ---- L4: per-core k-partial of out = W_out_slab.T @ h3_slab ----
            psums = [pmm.tile([128, B], f32, tag=f"mm{jt}", name=f"po{jt}")
                     for jt in range(8)]
            for kt in range(8):
                for jt in range(8):
                    nc.tensor.matmul(
                        psums[jt][:],
                        w_out[:, kt, jt * 128:(jt + 1) * 128],
                        h3[kt][:],
                        start=(kt == 0),
                        stop=(kt == 7),
                    )
            for jt in range(8):
                osb = apool.tile([128, B], f32, tag="out", name="osb")
                nc.vector.tensor_copy(osb[:], psums[jt][:])
                nc.gpsimd.dma_start(out=out_d[jt], in_=osb[:])

    nc.compile()
    return nc


def _perm():
    """k-order of the half-gathered activations: rank-major halves."""
    return np.concatenate(
        [np.arange(r * OSLAB + h * HALF, r * OSLAB + (h + 1) * HALF)
         for h in range(2) for r in range(NCORES)])


def _prep_inputs(x, W_in, b_in, W_mid, b_mid, W_out, b_out, indx_seqs):
    """Host-side compile-time transforms of inputs (per-core slabs)."""
    idx = np.asarray(indx_seqs).astype(np.int64)
    perm = _perm()

    def build_S(Wm):
        # S[k, o] = sum_f Wm[o, f] * [idx[o, f] == k], k rows permuted
        S = np.zeros((NUM_MID, NUM_MID), np.float32)
        cols = np.repeat(np.arange(NUM_MID), FAN_IN)
        np.add.at(S, (idx.reshape(-1), cols), np.asarray(Wm, np.float32).reshape(-1))
        return S[perm].reshape(64, 128, NUM_MID).astype(BF16)

    s1_t = build_S(W_mid[0])
    s2_t = build_S(W_mid[1])

    x = np.asarray(x, np.float32)
    xT = np.ascontiguousarray(x.T.reshape(8, 128, B).astype(BF16))
    w_in_t = np.asarray(W_in, np.float32).T.reshape(8, 128, NUM_MID).astype(BF16)
    woT = np.asarray(W_out, np.float32).T  # [8192, 1000]

    def bias_slab(b, c):
        return np.ascontiguousarray(
            np.asarray(b, np.float32)[c * OSLAB:(c + 1) * OSLAB].reshape(8, 128).T)

    in_maps = []
    for c in range(NCORES):
        sl = slice(c * OSLAB, (c + 1) * OSLAB)
        wo = np.zeros((OSLAB, 1024), np.float32)
        wo[:, :NUM_OUT] = woT[sl]
        in_maps.append({
            "xT": xT,
            "w_in": np.ascontiguousarray(w_in_t[:, :, sl]),
            "b_in": bias_slab(b_in, c),
            "s1": np.ascontiguousarray(s1_t[:, :, sl]),
            "b1": bias_slab(b_mid[0], c),
            "s2": np.ascontiguousarray(s2_t[:, :, sl]),
            "b2": bias_slab(b_mid[1], c),
            "w_out": np.ascontiguousarray(wo.reshape(8, 128, 1024).astype(BF16)),
        })
    return in_maps, np.asarray(b_out, np.float32)


def kernel(x, W_in, b_in, W_mid, b_mid, W_out, b_out, indx_seqs):
    global LAST_RESULT
    if "nc" not in _cache:
        _cache["nc"] = _build_nc()
    nc = _cache["nc"]

    in_maps, b_out_f = _prep_inputs(x, W_in, b_in, W_mid, b_mid, W_out, b_out,
                                    indx_seqs)

    res = bass_utils.run_bass_kernel_spmd(
        nc, in_maps, core_ids=list(range(NCORES)),
        trace=bool(int(os.environ.get("KERNEL_TRACE", "0"))),
    )
    LAST_RESULT = res

    acc = np.zeros((1024, B), np.float64)
    for r in res.results:
        acc += r["out"].reshape(1024, B)
    out = acc[:NUM_OUT].T + b_out_f[None, :]
    return np.ascontiguousarray(out).astype(np.float32)
